# revision 7
# baseline (speedup 1.0000x reference)
"""Trainium2 Bass kernel for nn_DCTFeatureModel.

Math: the reference pipeline (3D DCT-II over [time-in-bin, H, W], mean over
DCT bins, full-receptive-field Conv3d, bias, LeakyReLU) is linear up to the
LeakyReLU, so everything folds into a single small matmul:

    feat[b,s,o] = LeakyReLU( sum_{c,t,i,j} x[b,s,c,t,i,j] * Weff[s,o,t,i,j]
                             + bias[s,o] )
    Weff[s,o,t,i,j] = (1/8) * sum_{f,p,q} Ct[f,t] Cs[p,i] Cs[q,j] W[s,o,f,p,q]

Weff is tiny (2*64*2048 floats) and computed on host. The device kernel is
memory-bound: stream x, reduce over the 8 DCT bins (c), then a small matmul.

v4 device dataflow (per core): x is shipped as bf16 (halves HBM traffic; the
2e-2 rel-err budget dwarfs bf16 quantization), laid out as one contiguous
[128 kin, 2(h) x 16(ch) x 128(b)] HBM block per (s, bin-pair), streamed as
32 quarter-tiles of 256 KiB on two HWDGE engines (sync + scalar). Every x
buffer is dedicated (no recycling) and the ENTIRE 8-bin mean reduction rides
the PE's PSUM accumulation - 256 [128k x 128b x 64o] matmuls at ~53 ns each
consume quarters as they land, so compute fully overlaps the DMA stream and
no vector-engine adds exist. Rank-1 bias matmul closes each accumulation
group; LeakyReLU = max(v, 0.02v) on DVE; single 64 KiB out DMA.

Sharding: pure data-parallel over batch, 1024/8 = 128 rows per core.
"""

from contextlib import ExitStack

import ml_dtypes
import numpy as np

import concourse.bacc as bacc
import concourse.tile as tile
from concourse import mybir
from concourse.bass_utils import run_bass_kernel_spmd

# Problem shapes (hardcoded per contract)
B = 1024
NCORES = 8
BS = B // NCORES          # 128 batch rows per core
NSW = 2                   # subwindows
NBINS = 8                 # DCT bins (mean-reduced)
NPAIR = NBINS // 2        # 4 bin-pairs per subwindow
NDCT = 32                 # time points per bin
HW = 8
NF = 64                   # conv output filters per subwindow
K = NDCT * HW * HW        # 2048 contraction elements per (s, c)
P = 128                   # partitions
NCHUNK = K // P           # 16 k-chunks of 128
OUT_F = NSW * NF          # 128 output features
SLOPE = 0.02

F32 = mybir.dt.float32
BF16 = mybir.dt.bfloat16



_cached = None
last_results = None


def _dct2(N):
    n = np.arange(N, dtype=np.float64)
    k = np.arange(N, dtype=np.float64)
    return 2.0 * np.cos(np.pi * (2.0 * n[None, :] + 1.0) * k[:, None] / (2.0 * N))


def _kernel_body(tc, x, w, bias, out):
    """x: [NSW*NPAIR, 128, 2*NCHUNK*BS] bf16, one (s, pair) block per row,
    cols = (h, ch, b); w: [P, NSW*NCHUNK*NF] bf16; bias: [1, OUT_F] bf16;
    out: [BS, OUT_F] f32."""
    nc = tc.nc
    with ExitStack() as ctx:
        const_pool = ctx.enter_context(tc.tile_pool(name="const", bufs=1))
        upool = ctx.enter_context(tc.tile_pool(name="up", bufs=1))
        opool = ctx.enter_context(tc.tile_pool(name="op", bufs=1))
        pft_pool = ctx.enter_context(tc.tile_pool(name="pft", bufs=1, space="PSUM"))

        w_sb = const_pool.tile([P, NSW * NCHUNK * NF], BF16)
        nc.scalar.dma_start(out=w_sb, in_=w)
        bias_sb = const_pool.tile([1, OUT_F], BF16)
        nc.scalar.dma_start(out=bias_sb, in_=bias)
        ones = const_pool.tile([1, BS], BF16)
        nc.vector.memset(ones, 1.0)

        out_sb = opool.tile([BS, OUT_F], F32)
        psum_feat = [
            pft_pool.tile([BS, NF], F32, tag=f"feat{s}", name=f"psum_feat{s}")
            for s in range(NSW)
        ]

        # Stream x as 32 quarter-tiles of 256 KiB. Small transfers make tile
        # completions STAGGER (~0.9 us apart) instead of all landing at once
        # (8 concurrent 1 MiB DMAs round-robin the SDMA engines and finish
        # simultaneously, serializing all compute into a 13 us tail). The PE
        # consumes each quarter as it lands; the whole 8-bin mean rides PSUM
        # accumulation, so no vector-engine adds at all.
        QC = 8                 # chunks per quarter
        QW = QC * BS           # 1024 cols per quarter
        NQ = 4                 # quarters per (s, pair) block
        qtiles = {}
        for s in range(NSW):
            for m in range(NPAIR):
                for q in range(NQ):
                    t = upool.tile(
                        [P, QW], BF16, tag=f"x{s}_{m}_{q}", name=f"x_{s}_{m}_{q}"
                    )
                    qtiles[(s, m, q)] = t
        # sync carries pairs m=0,1; scalar (after w+bias) carries m=2,3;
        # both engines work s=0 before s=1 so PE's s-major order matches
        # arrival order.
        for s in range(NSW):
            for m in range(NPAIR):
                eng = nc.sync if m < 2 else nc.scalar
                for q in range(NQ):
                    eng.dma_start(
                        out=qtiles[(s, m, q)],
                        in_=x[s * NPAIR + m, :, q * QW:(q + 1) * QW],
                    )

        for s in range(NSW):
            for m in range(NPAIR):
                for q in range(NQ):
                    t = qtiles[(s, m, q)]
                    ch0 = (q % 2) * QC  # quarter q covers (h=q//2, chunks ch0..ch0+7)
                    for j in range(QC):
                        ch = ch0 + j
                        nc.tensor.matmul(
                            psum_feat[s],
                            lhsT=t[:, j * P:(j + 1) * P],
                            rhs=w_sb[:, (s * NCHUNK + ch) * NF:(s * NCHUNK + ch + 1) * NF],
                            start=(m == 0 and q == 0 and j == 0),
                            stop=False,
                        )
            nc.tensor.matmul(
                psum_feat[s],
                lhsT=ones,
                rhs=bias_sb[:, s * NF:(s + 1) * NF],
                start=False,
                stop=True,
            )
            # LeakyReLU(v) = max(v, slope*v), exact on DVE (scalar-engine
            # Lrelu is table-based and costs ~3x the error)
            tmp = upool.tile([BS, NF], F32, tag=f"lr{s}", name=f"lr_{s}")
            nc.vector.tensor_scalar_mul(tmp, psum_feat[s], SLOPE)
            nc.vector.tensor_max(
                out=out_sb[:, s * NF:(s + 1) * NF], in0=psum_feat[s], in1=tmp
            )

        nc.sync.dma_start(out=out, in_=out_sb)


def _build():
    global _cached
    if _cached is not None:
        return _cached
    nc = bacc.Bacc(
        "TRN2",
        target_bir_lowering=False,
        debug=False,
        enable_asserts=False,
        num_devices=NCORES,
    )
    x_ap = nc.dram_tensor(
        "x", [NSW * NPAIR, P, 2 * NCHUNK * BS], BF16, kind="ExternalInput"
    ).ap()
    w_ap = nc.dram_tensor("w", [P, NSW * NCHUNK * NF], BF16, kind="ExternalInput").ap()
    b_ap = nc.dram_tensor("bias", [1, OUT_F], BF16, kind="ExternalInput").ap()
    out_ap = nc.dram_tensor("out", [BS, OUT_F], F32, kind="ExternalOutput").ap()
    with tile.TileContext(nc, trace_sim=False) as tc:
        _kernel_body(tc, x_ap, w_ap, b_ap, out_ap)
    nc.compile()
    _cached = nc
    return nc


def kernel(x, W, b):
    global last_results
    assert x.shape == (B, 1, NSW * NBINS * NDCT, HW, HW), x.shape
    nc = _build()

    # Host-side folding of the DCT matrices into the conv weights (tiny).
    Ct = _dct2(NDCT)                       # [f, t]
    Cs = _dct2(HW)                         # [p, i]
    Weff = np.einsum(
        "ft,pi,qj,sofpq->sotij", Ct, Cs, Cs, W.astype(np.float64), optimize=True
    ) / float(NBINS)
    Weff_k = Weff.reshape(NSW, NF, K)      # [s, o, k]
    # device layout: w[p, s*NCHUNK*NF + ch*NF + o] = Weff_k[s, o, ch*128 + p]
    w_dev = np.ascontiguousarray(
        Weff_k.reshape(NSW, NF, NCHUNK, P).transpose(3, 0, 2, 1).reshape(P, NSW * NCHUNK * NF)
    ).astype(ml_dtypes.bfloat16)
    bias_dev = np.ascontiguousarray(b.reshape(1, OUT_F)).astype(ml_dtypes.bfloat16)

    # (b, s, m, h, ch, kin) with bin c = 2m + h
    x_bf = x.reshape(B, NSW, NPAIR, 2, NCHUNK, P).astype(ml_dtypes.bfloat16)
    in_maps = []
    for i in range(NCORES):
        xs = x_bf[i * BS:(i + 1) * BS]
        # -> [s, m, kin, h, ch, b]: per (s, m) two contiguous 512 KiB halves
        xt = np.ascontiguousarray(xs.transpose(1, 2, 5, 3, 4, 0)).reshape(
            NSW * NPAIR, P, 2 * NCHUNK * BS
        )
        in_maps.append({"x": xt, "w": w_dev, "bias": bias_dev})
    res = run_bass_kernel_spmd(nc, in_maps, core_ids=list(range(NCORES)))
    last_results = res
    return np.concatenate([r["out"] for r in res.results], axis=0)
